# revision 13
# baseline (speedup 1.0000x reference)
"""CTC loss (nn_CTC_28819230556189) on 8 Trainium2 NeuronCores via Bass/Tile.

Data-parallel over batch (4 examples/core). Per core:

  Phase 1 (PE + Act):  logits = hpad @ W.T in fp8 DoubleRow (K=256/matmul);
    lse side:  exp(logit - C) accumulated over V -> lsum[t]; ln(lsum)
               partition-reduced via a ones-matmul -> Sum_t ln lsum (llacc).
    glog side: the extended label sequence has only 101 distinct tokens per
               example (blank + 100 labels), so glog^T is computed as a
               [101, t] matmul; p~ = exp(glog + D) in bf16 is DMA-transposed
               into psweep[example, row, t] (row 0 = blank, row 1+j = label j).
    The -lse term is NOT folded into p~: every CTC path takes exactly one
    emission per frame, so ll = ln(sum B~) - Sum_t lse_t - T*D, with
    lse_t = ln lsum_t + C.

  Phase 2 (DVE): CTC forward DP restructured as an s-sweep: for each
    extended-label state s (201 of them), ONE tensor_tensor_scan over the
    frames computes  B_s(t) = (B_s(t-1) + v_s(t)) * p~_t[s]  with
    v_s = B_{s-1}(t-1) [+ B_{s-2}(t-1) at non-blank s, masked only at the
    rare repeated-label positions].  Scans are right-trimmed to the frames
    from which the terminal states remain reachable.  ~300 DVE ops total;
    scan state is fp32 internally.

  Loss partials summed on host (no collectives needed).
"""

import numpy as np

import concourse.bass as bass
import concourse.bacc as bacc
import concourse.tile as tile
import concourse.mybir as mybir
from concourse.bass_utils import run_bass_kernel_spmd

BF16 = mybir.dt.bfloat16
F32 = mybir.dt.float32
FP8 = mybir.dt.float8e4
AF = mybir.ActivationFunctionType
ALU = mybir.AluOpType
AX = mybir.AxisListType
DR = mybir.MatmulPerfMode.DoubleRow

# Problem shapes (hardcoded per spec nn_CTC_28819230556189)
B, T, E, V, L = 32, 500, 1024, 5000, 100
S = 2 * L + 1           # 201 extended labels
NCORE = 8
BPC = B // NCORE        # 4 examples per core
NPAIR = E // 256        # 4 double-row K-pairs (256 contraction each)
TC = 125                # time chunk
NCHUNK = T // TC        # 4
VC = 500                # v-chunk width (one PSUM bank in f32)
NV = V // VC            # 10
C_SHIFT = 4.0           # logsumexp constant shift (logits ~ N(0,1))
D_SHIFT = -1.1          # p~ = exp(glog + D); keeps ln(sum B~) drift ~ 0
NR = L + 1              # distinct p~ rows per example (blank + labels)
TCP = 128               # hp stationary pair-stride (16B-aligned pad of TC)
RP = 112                # wext stationary pair-stride (16B-aligned pad of NR)

_cache = {}


def _hi(s):
    """Last frame (inclusive) from which state s can still reach a terminal
    state ({S-2, S-1}) by frame T-1; B_s beyond it cannot contribute."""
    need = max(0, (S - 2) - s)
    return min(T - 1, T - 1 - (need + 1) // 2)


def _build_nc(masked_odd):
    """masked_odd: sorted tuple of odd s positions where some example in the
    batch has a repeated label (skip transition disallowed) -> those sweep
    iterations apply the per-example m2 mask; all other odd s use a plain
    add (mask == 1 for every example by construction)."""
    nc = bacc.Bacc("TRN2", target_bir_lowering=False, debug=False,
                   enable_asserts=False)

    for val in (-C_SHIFT, D_SHIFT):
        cth = nc.alloc_sbuf_tensor(f"const-f32-{val}", [128, 1], F32)
        nc.gpsimd.memset(cth.ap(), val)
        nc.const_aps.aps[(F32, val)] = cth.ap()
    nc.all_engine_barrier()

    hpt_d = nc.dram_tensor("hpt", [BPC, NPAIR, 2, 128, T], FP8,
                           kind="ExternalInput")
    wtt_d = nc.dram_tensor("wtt", [NPAIR, 2, 128, V], FP8,
                           kind="ExternalInput")
    wxt_d = nc.dram_tensor("wxt", [BPC, NPAIR, 2, 128, NR], FP8,
                           kind="ExternalInput")
    m2_d = nc.dram_tensor("m2", [BPC, S], F32, kind="ExternalInput")
    out_d = nc.dram_tensor("out", [1, 1], F32, kind="ExternalOutput")

    with tile.TileContext(nc) as tc:
      with tc.tile_pool(name="persist", bufs=1) as pers:
        def ptile(shape, dtype, nm):
            return pers.tile(shape, dtype, tag=nm, name=nm)

        # ---- resident weights ----
        # wt loads split by v-chunk so the first matmuls aren't gated on the
        # full 5 MB load; Tile's subtile deps let v-chunk 0 matmuls start
        # as soon as its slices land.
        wt_all = ptile([128, NPAIR, 2, V], FP8, "wt_all")
        for v in range(NV):
            for pe in range(NPAIR):
                for i in range(2):
                    nc.sync.dma_start(
                        wt_all[:, pe, i, v * VC:(v + 1) * VC],
                        wtt_d[pe, i, :, v * VC:(v + 1) * VC])
        wx_all = ptile([128, BPC, NPAIR, 2, RP], FP8, "wx_all")
        for bb in range(BPC):
            for pe in range(NPAIR):
                for i in range(2):
                    nc.sync.dma_start(wx_all[:, bb, pe, i, 0:NR],
                                      wxt_d[bb, pe, i])
        m2t = ptile([BPC, S], F32, "m2t")
        nc.sync.dma_start(m2t[:], m2_d[:])
        ones125 = ptile([125, 1], BF16, "ones125")
        nc.vector.memset(ones125[:], 1.0)
        # ln(lsum) per (example, chunk); reduced once at the end of phase 1
        # so no PE instruction ever waits mid-phase on the Act chain.
        lnls_all = ptile([125, BPC * NCHUNK], BF16, "lnls_all")

        # ---- sweep state ----
        psweep = ptile([BPC, NR, T], BF16, "psweep")
        brows = ptile([BPC, 3, T + 1], BF16, "brows")
        nc.vector.memset(brows[:], 0.0)
        nc.vector.memset(brows[:, 0, 0:1], 1.0)   # B_0(-1) = 1
        zrow = ptile([BPC, T], BF16, "zrow")
        nc.vector.memset(zrow[:], 0.0)
        vtmp = ptile([BPC, T], BF16, "vtmp")
        llacc = ptile([1, BPC], F32, "llacc")     # Sum_t ln lsum_t per ex
        nc.vector.memset(llacc[:], 0.0)

        with (
            tc.tile_pool(name="hp", bufs=2) as hp_pool,
            tc.tile_pool(name="scr", bufs=2) as scr_pool,
            tc.tile_pool(name="small", bufs=4) as small_pool,
            tc.tile_pool(name="pt", bufs=2) as pt_pool,
            tc.tile_pool(name="ps", bufs=2, space="PSUM") as ps_pool,
            tc.tile_pool(name="gl", bufs=2, space="PSUM") as gl_pool,
            tc.tile_pool(name="lsps", bufs=2, space="PSUM") as lsps_pool,
        ):
            # ================= Phase 1: matmuls / lse / p~ =================
            for c in range(NCHUNK):
                t0 = c * TC
                for bb in range(BPC):
                    hp_t = hp_pool.tile([128, NPAIR, 2, TCP], FP8, tag="hp",
                                        name="hp_t")
                    for pe in range(NPAIR):
                        for i in range(2):
                            nc.sync.dma_start(
                                hp_t[:, pe, i, 0:TC],
                                hpt_d[bb, pe, i, :, t0:t0 + TC])

                    spart = small_pool.tile([TC, NV // 2], F32, tag="spart",
                                            name="spart")
                    for k in range(NV // 2):
                        # two v-chunks -> two PSUM banks, one paired Act op
                        ps = ps_pool.tile([TC, 2, 512], F32, tag="ps",
                                          name="ps")
                        for h in range(2):
                            v = 2 * k + h
                            for pe in range(NPAIR):
                                nc.tensor.matmul(
                                    ps[:, h, 0:VC],
                                    hp_t[:, pe, :, 0:TC],
                                    wt_all[:, pe, :, v * VC:(v + 1) * VC],
                                    start=(pe == 0), stop=(pe == NPAIR - 1),
                                    perf_mode=DR)
                        scr = scr_pool.tile([TC, 2, VC], BF16, tag="scr",
                                            name="scr")
                        nc.scalar.activation(scr[:], ps[:, :, 0:VC], AF.Exp,
                                             bias=-C_SHIFT, scale=1.0,
                                             accum_out=spart[:, k:k + 1])
                    scr10 = small_pool.tile([TC, NV // 2], BF16, tag="scr10",
                                            name="scr10")
                    lsum = small_pool.tile([TC, 1], F32, tag="lsum",
                                           name="lsum")
                    nc.scalar.activation(scr10[:], spart[:], AF.Identity,
                                         accum_out=lsum[:])
                    col = bb * NCHUNK + c
                    nc.scalar.activation(lnls_all[:, col:col + 1], lsum[:],
                                         AF.Ln)

                    # glog^T [token-row, t], then p~ = exp(glog + D) in bf16
                    gl = gl_pool.tile([NR, TC], F32, tag="gl", name="gl")
                    for pe in range(NPAIR):
                        nc.tensor.matmul(
                            gl[:], wx_all[:, bb, pe, :, 0:NR],
                            hp_t[:, pe, :, 0:TC],
                            start=(pe == 0), stop=(pe == NPAIR - 1),
                            perf_mode=DR)
                    ptc = pt_pool.tile([NR, TC], BF16, tag="ptc", name="ptc")
                    nc.scalar.activation(ptc[:], gl[:], AF.Exp,
                                         bias=D_SHIFT, scale=1.0)
                    nc.sync.dma_start(psweep[bb:bb + 1, :, t0:t0 + TC],
                                      ptc[:])

            # Sum_t ln lsum: one ones-matmul partition reduce for all
            # (example, chunk) columns, then a per-example strided reduce.
            lsps = lsps_pool.tile([1, BPC, NCHUNK], F32, tag="lsps",
                                  name="lsps")
            nc.tensor.matmul(lsps[:], ones125[:], lnls_all[:],
                             start=True, stop=True)
            nc.vector.tensor_reduce(llacc[:], lsps[:], axis=AX.X, op=ALU.add)

            # ================= Phase 2: s-sweep DP =================
            masked = set(masked_odd)
            for s in range(S):
                hi = _hi(s)
                w = hi + 1                      # frames [0, hi]
                row = brows[:, s % 3, :]
                out_ap = row[:, 1:w + 1]
                prow = psweep[:, 0 if s % 2 == 0 else 1 + (s - 1) // 2, 0:w]
                if s == 3:
                    # B_0(-1)=1 was consumed by s=1; row 0 now recycles as
                    # B_3 whose halo must read 0 for s=4/s=5.
                    nc.vector.memset(brows[:, 0, 0:1], 0.0)
                if s == 0:
                    nc.vector.tensor_tensor_scan(
                        out_ap, zrow[:, 0:w], prow, 1.0, ALU.add, ALU.mult)
                    continue
                b1 = brows[:, (s - 1) % 3, 0:w]
                if s == 1 or s % 2 == 0:
                    # v = B_{s-1}(t-1) only (blank, or s=1 which has no s-2)
                    nc.vector.tensor_tensor_scan(
                        out_ap, b1, prow, 0.0, ALU.add, ALU.mult)
                    continue
                b2 = brows[:, (s - 2) % 3, 0:w]
                if s in masked:
                    nc.vector.tensor_scalar_mul(vtmp[:, 0:w], b2,
                                                m2t[:, s:s + 1])
                    nc.vector.tensor_add(vtmp[:, 0:w], vtmp[:, 0:w], b1)
                else:
                    nc.vector.tensor_add(vtmp[:, 0:w], b1, b2)
                nc.vector.tensor_tensor_scan(
                    out_ap, vtmp[:, 0:w], prow, 0.0, ALU.add, ALU.mult)

            # ================= finalize =================
            u = ptile([BPC, 1], F32, "u")
            nc.vector.tensor_add(u[:], brows[:, 200 % 3, T:T + 1],
                                 brows[:, 199 % 3, T:T + 1])
            lnu = ptile([BPC, 1], F32, "lnu")
            nc.scalar.activation(lnu[:], u[:], AF.Ln)
            llf = ptile([1, BPC], F32, "llf")
            nc.sync.dma_start(llf[:], lnu[:])   # [4,1] -> [1,4]
            dif = ptile([1, BPC], F32, "dif")
            nc.vector.tensor_tensor(dif[:], llf[:], llacc[:], ALU.subtract)
            tot = ptile([1, 1], F32, "tot")
            nc.vector.tensor_reduce(tot[:], dif[:], axis=AX.X, op=ALU.add)
            nc.sync.dma_start(out_d[:], tot[:])

    nc.compile()
    return nc


def prep_in_maps(hpad, W, b, ys):
    """Host-side layout prep shared by kernel() and test harnesses."""
    f8 = mybir.dt.np(FP8)
    W = np.asarray(W)
    ext = np.zeros((B, S), dtype=np.int64)
    ext[:, 1::2] = ys
    prev2 = np.full((B, S), -1, dtype=np.int64)
    prev2[:, 2:] = ext[:, :-2]
    allow2 = (ext != 0) & (ext != prev2)
    masked_odd = tuple(sorted(
        s for s in range(3, S, 2) if not allow2[:, s].all()))
    m2 = allow2.astype(np.float32)

    hpT = np.ascontiguousarray(hpad.transpose(0, 2, 1)).astype(f8)
    hpT = hpT.reshape(B, NPAIR, 2, 128, T)
    wtT = np.ascontiguousarray(W.T).astype(f8).reshape(NPAIR, 2, 128, V)
    # distinct tokens per example: row 0 = blank, row 1+j = label j
    toks = np.concatenate([np.zeros((B, 1), np.int64), np.asarray(ys, np.int64)],
                          axis=1)                       # [B, NR]
    wext = np.ascontiguousarray(
        W[toks.reshape(-1)].reshape(B, NR, E).transpose(0, 2, 1)
    ).astype(f8).reshape(B, NPAIR, 2, 128, NR)

    in_maps = []
    for c in range(NCORE):
        sl = slice(c * BPC, (c + 1) * BPC)
        in_maps.append({
            "hpt": np.ascontiguousarray(hpT[sl]),
            "wtt": wtT,
            "wxt": np.ascontiguousarray(wext[sl]),
            "m2": np.ascontiguousarray(m2[sl]),
        })
    return in_maps, masked_odd


def kernel(hpad, W, b, ys):
    assert hpad.shape == (B, T, E) and W.shape == (V, E) and ys.shape == (B, L)
    assert not np.any(np.asarray(b)), "kernel assumes b == 0 (per problem spec)"

    in_maps, masked_odd = prep_in_maps(hpad, W, b, ys)
    key = ("nc", masked_odd)
    if key not in _cache:
        _cache[key] = _build_nc(masked_odd)
    nc = _cache[key]
    _cache["nc_last"] = (nc, in_maps)

    res = run_bass_kernel_spmd(nc, in_maps, core_ids=list(range(NCORE)))
    tot = sum(float(r["out"][0, 0]) for r in res.results)
    ll_sum = tot - B * T * (C_SHIFT + D_SHIFT)
    return np.float32(-ll_sum / B)


# revision 14
# speedup vs baseline: 1.0923x; 1.0923x over previous
"""CTC loss (nn_CTC_28819230556189) on 8 Trainium2 NeuronCores via Bass/Tile.

Data-parallel over batch (4 examples/core). Per core:

  Phase 1 (PE + Act):  logits = hpad @ W.T in fp8 DoubleRow (K=256/matmul);
    lse side:  exp(logit - C) accumulated over V -> lsum[t]; ln(lsum) columns
               collected and partition-reduced ONCE at the end via a
               ones-matmul -> Sum_t ln lsum per example (llacc).
    glog side: the extended label sequence has only 101 distinct tokens per
               example (blank + 100 labels), so glog^T is computed as a
               [101, t] matmul; p~ = exp(glog + D) in bf16 is DMA-transposed
               into psweep[example, row, t] (row 0 = blank, row 1+j = label j).
    The -lse term is NOT folded into p~: every CTC path takes exactly one
    emission per frame, so ll = ln(sum B~) - Sum_t lse_t - T*D, with
    lse_t = ln lsum_t + C.

  Phase 2 (DVE): CTC forward DP as a label-sweep of pure affine scans
    (tensor_tensor_scan, fp32 internal state).  With E_j = alpha[blank 2j],
    O_j = alpha[label 2j+1], and F_j = E_j + allow_j * O_{j-1}:
        F_j(t) = q(t)*F_j(t-1) + O_{j-1}(t)        (scan: mult, add)
        O_j(t) = (O_j(t-1) + F_j(t-1)) * p_j(t)    (scan: add, mult)
    and the likelihood is simply F_L(T-1) = alpha_T[S-1] + alpha_T[S-2].
    201 scans total, no elementwise adds; each scan runs only over the
    frame window from which terminal states remain reachable (~402 of 500).
    Repeated labels (skip disallowed) get a rare blended-input fixup.

  Loss partials summed on host (no collectives needed).
"""

import numpy as np

import concourse.bass as bass
import concourse.bacc as bacc
import concourse.tile as tile
import concourse.mybir as mybir
from concourse.bass_utils import run_bass_kernel_spmd

BF16 = mybir.dt.bfloat16
F32 = mybir.dt.float32
FP8 = mybir.dt.float8e4
AF = mybir.ActivationFunctionType
ALU = mybir.AluOpType
AX = mybir.AxisListType
DR = mybir.MatmulPerfMode.DoubleRow

# Problem shapes (hardcoded per spec nn_CTC_28819230556189)
B, T, E, V, L = 32, 500, 1024, 5000, 100
S = 2 * L + 1           # 201 extended labels
NCORE = 8
BPC = B // NCORE        # 4 examples per core
NPAIR = E // 256        # 4 double-row K-pairs (256 contraction each)
TC = 125                # time chunk
NCHUNK = T // TC        # 4
VC = 500                # v-chunk width (one PSUM bank in f32)
NV = V // VC            # 10
C_SHIFT = 4.0           # logsumexp constant shift (logits ~ N(0,1))
D_SHIFT = -1.1          # p~ = exp(glog + D); keeps ln(sum B~) drift ~ 0
NTOK = L + 1            # distinct p~ rows per example (blank + labels)
TCP = 128               # hp stationary pair-stride (16B-aligned pad of TC)
RP = 112                # wext stationary pair-stride (16B-aligned pad of NTOK)

_cache = {}


def _hi(s):
    """Last frame (inclusive) from which extended state s can still reach a
    terminal state ({S-2, S-1}) by frame T-1."""
    need = max(0, (S - 2) - s)
    return min(T - 1, T - 1 - (need + 1) // 2)


def _build_nc(masked_j):
    """masked_j: sorted tuple of label indices j (1..L-1) where some example
    has ys[j] == ys[j-1] (skip transition disallowed) -> those F_j scans get
    a per-example blended input; all other j use O_{j-1} directly."""
    nc = bacc.Bacc("TRN2", target_bir_lowering=False, debug=False,
                   enable_asserts=False)

    for val in (-C_SHIFT, D_SHIFT):
        cth = nc.alloc_sbuf_tensor(f"const-f32-{val}", [128, 1], F32)
        nc.gpsimd.memset(cth.ap(), val)
        nc.const_aps.aps[(F32, val)] = cth.ap()
    nc.all_engine_barrier()

    hpt_d = nc.dram_tensor("hpt", [BPC, NPAIR, 2, 128, T], FP8,
                           kind="ExternalInput")
    wtt_d = nc.dram_tensor("wtt", [NPAIR, 2, 128, V], FP8,
                           kind="ExternalInput")
    wxt_d = nc.dram_tensor("wxt", [BPC, NPAIR, 2, 128, NTOK], FP8,
                           kind="ExternalInput")
    # m2[:, 0, j] = allow skip into label j; m2[:, 1, j] = 1 - that
    m2_d = nc.dram_tensor("m2", [BPC, 2, NTOK], F32, kind="ExternalInput")
    out_d = nc.dram_tensor("out", [1, 1], F32, kind="ExternalOutput")

    with tile.TileContext(nc) as tc:
      with tc.tile_pool(name="persist", bufs=1) as pers:
        def ptile(shape, dtype, nm):
            return pers.tile(shape, dtype, tag=nm, name=nm)

        wt_all = ptile([128, NPAIR, 2, V], FP8, "wt_all")
        wx_all = ptile([128, BPC, NPAIR, 2, RP], FP8, "wx_all")
        m2t = ptile([BPC, 2, NTOK], F32, "m2t")

        with (
            tc.tile_pool(name="hp", bufs=4) as hp_pool,
            tc.tile_pool(name="scr", bufs=2) as scr_pool,
            tc.tile_pool(name="small", bufs=4) as small_pool,
            tc.tile_pool(name="pt", bufs=2) as pt_pool,
            tc.tile_pool(name="ps", bufs=2, space="PSUM") as ps_pool,
            tc.tile_pool(name="gl", bufs=2, space="PSUM") as gl_pool,
            tc.tile_pool(name="lsps", bufs=1, space="PSUM") as lsps_pool,
        ):
            # ---- DMA issue order is the phase-1 critical path: chunk-0
            # activations and the first two v-slices of W go first so the
            # first matmul isn't gated on the full 8 MB of weights.
            hp_c0 = []
            for bb in range(BPC):
                hp_t = hp_pool.tile([128, NPAIR, 2, TCP], FP8, tag="hp",
                                    name="hp_t")
                for pe in range(NPAIR):
                    for i in range(2):
                        nc.sync.dma_start(hp_t[:, pe, i, 0:TC],
                                          hpt_d[bb, pe, i, :, 0:TC])
                hp_c0.append(hp_t)
            for v in (0, 1):
                for pe in range(NPAIR):
                    for i in range(2):
                        nc.sync.dma_start(
                            wt_all[:, pe, i, v * VC:(v + 1) * VC],
                            wtt_d[pe, i, :, v * VC:(v + 1) * VC])
            for bb in range(BPC):
                for pe in range(NPAIR):
                    for i in range(2):
                        nc.sync.dma_start(wx_all[:, bb, pe, i, 0:NTOK],
                                          wxt_d[bb, pe, i])
            nc.sync.dma_start(m2t[:], m2_d[:])
            for v in range(2, NV):
                for pe in range(NPAIR):
                    for i in range(2):
                        nc.sync.dma_start(
                            wt_all[:, pe, i, v * VC:(v + 1) * VC],
                            wtt_d[pe, i, :, v * VC:(v + 1) * VC])

            ones125 = ptile([125, 1], BF16, "ones125")
            nc.vector.memset(ones125[:], 1.0)
            lnls_all = ptile([125, BPC * NCHUNK], BF16, "lnls_all")
            llacc = ptile([1, BPC], F32, "llacc")

            # ---- sweep state ----
            psweep = ptile([BPC, NTOK, T], BF16, "psweep")
            frow = ptile([BPC, T + 1], BF16, "frow")   # col k = F(t=k-1)
            orow = ptile([BPC, 2, T], BF16, "orow")    # col k = O(t=k)
            nc.vector.memset(frow[:], 0.0)
            nc.vector.memset(frow[:, 0:1], 1.0)        # F_0(-1) = B_0(-1) = 1
            nc.vector.memset(orow[:], 0.0)
            zrow = ptile([BPC, T], BF16, "zrow")
            nc.vector.memset(zrow[:], 0.0)
            vtmp = ptile([BPC, T], BF16, "vtmp")
            vtmp2 = ptile([BPC, T], BF16, "vtmp2")

            # ================= Phase 1: matmuls / lse / p~ =================
            for c in range(NCHUNK):
                t0 = c * TC
                for bb in range(BPC):
                    if c == 0:
                        hp_t = hp_c0[bb]
                    else:
                        hp_t = hp_pool.tile([128, NPAIR, 2, TCP], FP8,
                                            tag="hp", name="hp_t")
                        for pe in range(NPAIR):
                            for i in range(2):
                                nc.sync.dma_start(
                                    hp_t[:, pe, i, 0:TC],
                                    hpt_d[bb, pe, i, :, t0:t0 + TC])

                    spart = small_pool.tile([TC, NV // 2], F32, tag="spart",
                                            name="spart")
                    for k in range(NV // 2):
                        # two v-chunks -> two PSUM banks, one paired Act op
                        ps = ps_pool.tile([TC, 2, 512], F32, tag="ps",
                                          name="ps")
                        for h in range(2):
                            v = 2 * k + h
                            for pe in range(NPAIR):
                                nc.tensor.matmul(
                                    ps[:, h, 0:VC],
                                    hp_t[:, pe, :, 0:TC],
                                    wt_all[:, pe, :, v * VC:(v + 1) * VC],
                                    start=(pe == 0), stop=(pe == NPAIR - 1),
                                    perf_mode=DR)
                        scr = scr_pool.tile([TC, 2, VC], BF16, tag="scr",
                                            name="scr")
                        nc.scalar.activation(scr[:], ps[:, :, 0:VC], AF.Exp,
                                             bias=-C_SHIFT, scale=1.0,
                                             accum_out=spart[:, k:k + 1])
                    scr10 = small_pool.tile([TC, NV // 2], BF16, tag="scr10",
                                            name="scr10")
                    lsum = small_pool.tile([TC, 1], F32, tag="lsum",
                                           name="lsum")
                    nc.scalar.activation(scr10[:], spart[:], AF.Identity,
                                         accum_out=lsum[:])
                    col = bb * NCHUNK + c
                    nc.scalar.activation(lnls_all[:, col:col + 1], lsum[:],
                                         AF.Ln)

                    # glog^T [token-row, t], then p~ = exp(glog + D) in bf16
                    gl = gl_pool.tile([NTOK, TC], F32, tag="gl", name="gl")
                    for pe in range(NPAIR):
                        nc.tensor.matmul(
                            gl[:], wx_all[:, bb, pe, :, 0:NTOK],
                            hp_t[:, pe, :, 0:TC],
                            start=(pe == 0), stop=(pe == NPAIR - 1),
                            perf_mode=DR)
                    ptc = pt_pool.tile([NTOK, TC], BF16, tag="ptc",
                                       name="ptc")
                    nc.scalar.activation(ptc[:], gl[:], AF.Exp,
                                         bias=D_SHIFT, scale=1.0)
                    nc.sync.dma_start(psweep[bb:bb + 1, :, t0:t0 + TC],
                                      ptc[:])

            # Sum_t ln lsum: one ones-matmul partition reduce for all
            # (example, chunk) columns, then a per-example strided reduce.
            lsps = lsps_pool.tile([1, BPC, NCHUNK], F32, tag="lsps",
                                  name="lsps")
            nc.tensor.matmul(lsps[:], ones125[:], lnls_all[:],
                             start=True, stop=True)
            nc.vector.tensor_reduce(llacc[:], lsps[:], axis=AX.X, op=ALU.add)

            # ================= Phase 2: F/O scan sweep =================
            masked = set(masked_j)
            qrow = psweep[:, 0, :]          # blank p~ row
            for j in range(L + 1):
                # F_j over frames [loF, hiF]; true support starts at j-1
                loF = max(0, j - 1)
                hiF = _hi(2 * j)
                if j == 0:
                    nc.vector.tensor_tensor_scan(
                        frow[:, 1:hiF + 2], qrow[:, 0:hiF + 1],
                        zrow[:, 0:hiF + 1], 1.0, ALU.mult, ALU.add)
                else:
                    oprev = orow[:, (j - 1) % 2, :]
                    if j in masked:
                        # d1 = m2*O_{j-1}(t) + (1-m2)*q(t)*O_{j-1}(t-1)
                        nc.vector.memset(vtmp[:, loF:loF + 1], 0.0)
                        nc.vector.tensor_mul(vtmp[:, loF + 1:hiF + 1],
                                             qrow[:, loF + 1:hiF + 1],
                                             oprev[:, loF:hiF])
                        nc.vector.tensor_scalar_mul(vtmp[:, loF:hiF + 1],
                                                    vtmp[:, loF:hiF + 1],
                                                    m2t[:, 1, j:j + 1])
                        nc.vector.tensor_scalar_mul(vtmp2[:, loF:hiF + 1],
                                                    oprev[:, loF:hiF + 1],
                                                    m2t[:, 0, j:j + 1])
                        nc.vector.tensor_add(vtmp[:, loF:hiF + 1],
                                             vtmp[:, loF:hiF + 1],
                                             vtmp2[:, loF:hiF + 1])
                        d1 = vtmp[:, loF:hiF + 1]
                    else:
                        d1 = oprev[:, loF:hiF + 1]
                    nc.vector.tensor_tensor_scan(
                        frow[:, loF + 1:hiF + 2], qrow[:, loF:hiF + 1],
                        d1, 0.0, ALU.mult, ALU.add)
                if j == L:
                    break
                loO = j
                hiO = _hi(2 * j + 1)
                nc.vector.tensor_tensor_scan(
                    orow[:, j % 2, loO:hiO + 1], frow[:, loO:hiO + 1],
                    psweep[:, 1 + j, loO:hiO + 1], 0.0, ALU.add, ALU.mult)

            # ================= finalize: ll = ln F_L(T-1) - llacc ==========
            lnu = ptile([BPC, 1], F32, "lnu")
            nc.scalar.activation(lnu[:], frow[:, T:T + 1], AF.Ln)
            llf = ptile([1, BPC], F32, "llf")
            nc.sync.dma_start(llf[:], lnu[:])   # [4,1] -> [1,4]
            dif = ptile([1, BPC], F32, "dif")
            nc.vector.tensor_tensor(dif[:], llf[:], llacc[:], ALU.subtract)
            tot = ptile([1, 1], F32, "tot")
            nc.vector.tensor_reduce(tot[:], dif[:], axis=AX.X, op=ALU.add)
            nc.sync.dma_start(out_d[:], tot[:])

    nc.compile()
    return nc


def prep_in_maps(hpad, W, b, ys):
    """Host-side layout prep shared by kernel() and test harnesses."""
    f8 = mybir.dt.np(FP8)
    W = np.asarray(W)
    ys = np.asarray(ys)
    # allow skip into label j (j >= 1): labels differ; j=0 has no skip source
    allow = np.ones((B, NTOK), np.float32)
    allow[:, 1:L] = (ys[:, 1:] != ys[:, :-1]).astype(np.float32)
    masked_j = tuple(sorted(
        j for j in range(1, L) if not allow[:, j].all()))
    m2 = np.stack([allow, 1.0 - allow], axis=1)       # [B, 2, NTOK]

    hpT = np.ascontiguousarray(hpad.transpose(0, 2, 1)).astype(f8)
    hpT = hpT.reshape(B, NPAIR, 2, 128, T)
    wtT = np.ascontiguousarray(W.T).astype(f8).reshape(NPAIR, 2, 128, V)
    # distinct tokens per example: row 0 = blank, row 1+j = label j
    toks = np.concatenate([np.zeros((B, 1), np.int64),
                           ys.astype(np.int64)], axis=1)   # [B, NTOK]
    wext = np.ascontiguousarray(
        W[toks.reshape(-1)].reshape(B, NTOK, E).transpose(0, 2, 1)
    ).astype(f8).reshape(B, NPAIR, 2, 128, NTOK)

    in_maps = []
    for c in range(NCORE):
        sl = slice(c * BPC, (c + 1) * BPC)
        in_maps.append({
            "hpt": np.ascontiguousarray(hpT[sl]),
            "wtt": wtT,
            "wxt": np.ascontiguousarray(wext[sl]),
            "m2": np.ascontiguousarray(m2[sl]),
        })
    return in_maps, masked_j


def kernel(hpad, W, b, ys):
    assert hpad.shape == (B, T, E) and W.shape == (V, E) and ys.shape == (B, L)
    assert not np.any(np.asarray(b)), "kernel assumes b == 0 (per problem spec)"

    in_maps, masked_j = prep_in_maps(hpad, W, b, ys)
    key = ("nc", masked_j)
    if key not in _cache:
        _cache[key] = _build_nc(masked_j)
    nc = _cache[key]
    _cache["nc_last"] = (nc, in_maps)

    res = run_bass_kernel_spmd(nc, in_maps, core_ids=list(range(NCORE)))
    tot = sum(float(r["out"][0, 0]) for r in res.results)
    ll_sum = tot - B * T * (C_SHIFT + D_SHIFT)
    return np.float32(-ll_sum / B)


# revision 15
# speedup vs baseline: 1.2312x; 1.1272x over previous
"""CTC loss (nn_CTC_28819230556189) on 8 Trainium2 NeuronCores via Bass/Tile.

Data-parallel over batch (4 examples/core). Per core:

  Phase 1 (PE + Act):  logits = hpad @ W.T in fp8 DoubleRow (K=256/matmul);
    lse side:  exp(logit - C) accumulated over V -> lsum[t]; ln(lsum) columns
               collected and partition-reduced ONCE at the end via a
               ones-matmul -> Sum_t ln lsum per example (llacc).
    glog side: the extended label sequence has only 101 distinct tokens per
               example (blank + 100 labels), so glog^T is computed as a
               [101, t] matmul; p~ = exp(glog + D) in bf16 is DMA-transposed
               into psweep[example, row, t] (row 0 = blank, row 1+j = label j).
    The -lse term is NOT folded into p~: every CTC path takes exactly one
    emission per frame, so ll = ln(sum B~) - Sum_t lse_t - T*D, with
    lse_t = ln lsum_t + C.

  Phase 2 (DVE): CTC forward DP as a label-sweep of pure affine scans
    (tensor_tensor_scan, fp32 internal state).  With E_j = alpha[blank 2j],
    O_j = alpha[label 2j+1], and F_j = E_j + allow_j * O_{j-1}:
        F_j(t) = q(t)*F_j(t-1) + O_{j-1}(t)        (scan: mult, add)
        O_j(t) = (O_j(t-1) + F_j(t-1)) * p_j(t)    (scan: add, mult)
    and the likelihood is simply F_L(T-1) = alpha_T[S-1] + alpha_T[S-2].
    201 scans total, no elementwise adds; each scan runs only over the
    frame window from which terminal states remain reachable (~402 of 500).
    Repeated labels (skip disallowed) get a rare blended-input fixup.

  Loss partials summed on host (no collectives needed).
"""

import numpy as np

import concourse.bass as bass
import concourse.bacc as bacc
import concourse.tile as tile
import concourse.mybir as mybir
from concourse.bass_utils import run_bass_kernel_spmd

BF16 = mybir.dt.bfloat16
F32 = mybir.dt.float32
FP8 = mybir.dt.float8e4
AF = mybir.ActivationFunctionType
ALU = mybir.AluOpType
AX = mybir.AxisListType
DR = mybir.MatmulPerfMode.DoubleRow

# Problem shapes (hardcoded per spec nn_CTC_28819230556189)
B, T, E, V, L = 32, 500, 1024, 5000, 100
S = 2 * L + 1           # 201 extended labels
NCORE = 8
BPC = B // NCORE        # 4 examples per core
NPAIR = E // 256        # 4 double-row K-pairs (256 contraction each)
TC = 125                # time chunk
NCHUNK = T // TC        # 4
VC = 500                # v-chunk width (one PSUM bank in f32)
NV = V // VC            # 10
C_SHIFT = 4.0           # logsumexp constant shift (logits ~ N(0,1))
D_SHIFT = -1.1          # p~ = exp(glog + D); keeps ln(sum B~) drift ~ 0
NTOK = L + 1            # distinct p~ rows per example (blank + labels)
TCP = 128               # hp stationary pair-stride (16B-aligned pad of TC)
RP = 112                # wext stationary pair-stride (16B-aligned pad of NTOK)

_cache = {}


def _hi(s):
    """Last frame (inclusive) from which extended state s can still reach a
    terminal state ({S-2, S-1}) by frame T-1."""
    need = max(0, (S - 2) - s)
    return min(T - 1, T - 1 - (need + 1) // 2)


def _build_nc(masked_j):
    """masked_j: sorted tuple of label indices j (1..L-1) where some example
    has ys[j] == ys[j-1] (skip transition disallowed) -> those F_j scans get
    a per-example blended input; all other j use O_{j-1} directly."""
    nc = bacc.Bacc("TRN2", target_bir_lowering=False, debug=False,
                   enable_asserts=False)

    for val in (-C_SHIFT, D_SHIFT):
        cth = nc.alloc_sbuf_tensor(f"const-f32-{val}", [128, 1], F32)
        nc.gpsimd.memset(cth.ap(), val)
        nc.const_aps.aps[(F32, val)] = cth.ap()
    nc.all_engine_barrier()

    hpt_d = nc.dram_tensor("hpt", [BPC, NCHUNK, 128, NPAIR * 2 * TCP],
                           FP8, kind="ExternalInput")
    wtt_d = nc.dram_tensor("wtt", [NPAIR, 2, 128, V], FP8,
                           kind="ExternalInput")
    wxt_d = nc.dram_tensor("wxt", [BPC, NPAIR, 2, 128, NTOK], FP8,
                           kind="ExternalInput")
    # m2[:, 0, j] = allow skip into label j; m2[:, 1, j] = 1 - that
    m2_d = nc.dram_tensor("m2", [BPC, 2, NTOK], F32, kind="ExternalInput")
    out_d = nc.dram_tensor("out", [1, 1], F32, kind="ExternalOutput")

    with tile.TileContext(nc) as tc:
      with tc.tile_pool(name="persist", bufs=1) as pers:
        def ptile(shape, dtype, nm):
            return pers.tile(shape, dtype, tag=nm, name=nm)

        wt_all = ptile([128, NPAIR, 2, V], FP8, "wt_all")
        wx_all = ptile([128, BPC, NPAIR, 2, RP], FP8, "wx_all")
        m2t = ptile([BPC, 2, NTOK], F32, "m2t")

        with (
            tc.tile_pool(name="hp", bufs=8) as hp_pool,
            tc.tile_pool(name="scr", bufs=2) as scr_pool,
            tc.tile_pool(name="small", bufs=4) as small_pool,
            tc.tile_pool(name="pt", bufs=2) as pt_pool,
            tc.tile_pool(name="ps", bufs=2, space="PSUM") as ps_pool,
            tc.tile_pool(name="gl", bufs=2, space="PSUM") as gl_pool,
            tc.tile_pool(name="lsps", bufs=1, space="PSUM") as lsps_pool,
        ):
            # ---- DMA issue order is the phase-1 critical path: chunk-0
            # activations and the first two v-slices of W go first so the
            # first matmul isn't gated on the full 8 MB of weights.
            hp_c0 = []
            for bb in range(BPC):
                hp_t = hp_pool.tile([128, NPAIR, 2, TCP], FP8, tag="hp",
                                    name="hp_t")
                nc.sync.dma_start(hp_t[:], hpt_d[bb, 0])
                hp_c0.append(hp_t)
            for pe in range(NPAIR):
                for i in range(2):
                    nc.sync.dma_start(wt_all[:, pe, i, 0:2 * VC],
                                      wtt_d[pe, i, :, 0:2 * VC])
            for bb in range(BPC):
                for pe in range(NPAIR):
                    for i in range(2):
                        nc.sync.dma_start(wx_all[:, bb, pe, i, 0:NTOK],
                                          wxt_d[bb, pe, i])
            nc.sync.dma_start(m2t[:], m2_d[:])
            for v in range(2, NV, 2):
                for pe in range(NPAIR):
                    for i in range(2):
                        nc.sync.dma_start(
                            wt_all[:, pe, i, v * VC:(v + 2) * VC],
                            wtt_d[pe, i, :, v * VC:(v + 2) * VC])

            ones125 = ptile([125, 1], BF16, "ones125")
            nc.vector.memset(ones125[:], 1.0)
            lnls_all = ptile([125, BPC * NCHUNK], BF16, "lnls_all")
            llacc = ptile([1, BPC], F32, "llacc")

            # ---- sweep state ----
            psweep = ptile([BPC, NTOK, T], BF16, "psweep")
            frow = ptile([BPC, T + 1], BF16, "frow")   # col k = F(t=k-1)
            orow = ptile([BPC, 2, T], BF16, "orow")    # col k = O(t=k)
            nc.vector.memset(frow[:], 0.0)
            nc.vector.memset(frow[:, 0:1], 1.0)        # F_0(-1) = B_0(-1) = 1
            nc.vector.memset(orow[:], 0.0)
            zrow = ptile([BPC, T], BF16, "zrow")
            nc.vector.memset(zrow[:], 0.0)
            vtmp = ptile([BPC, T], BF16, "vtmp")
            vtmp2 = ptile([BPC, T], BF16, "vtmp2")

            # ================= Phase 1: matmuls / lse / p~ =================
            for c in range(NCHUNK):
                t0 = c * TC
                for bb in range(BPC):
                    if c == 0:
                        hp_t = hp_c0[bb]
                    else:
                        hp_t = hp_pool.tile([128, NPAIR, 2, TCP], FP8,
                                            tag="hp", name="hp_t")
                        nc.sync.dma_start(hp_t[:], hpt_d[bb, c])

                    spart = small_pool.tile([TC, NV // 2], F32, tag="spart",
                                            name="spart")
                    for k in range(NV // 2):
                        # two v-chunks -> two PSUM banks, one paired Act op
                        ps = ps_pool.tile([TC, 2, 512], F32, tag="ps",
                                          name="ps")
                        for h in range(2):
                            v = 2 * k + h
                            for pe in range(NPAIR):
                                nc.tensor.matmul(
                                    ps[:, h, 0:VC],
                                    hp_t[:, pe, :, 0:TC],
                                    wt_all[:, pe, :, v * VC:(v + 1) * VC],
                                    start=(pe == 0), stop=(pe == NPAIR - 1),
                                    perf_mode=DR)
                        scr = scr_pool.tile([TC, 2, VC], BF16, tag="scr",
                                            name="scr")
                        nc.scalar.activation(scr[:], ps[:, :, 0:VC], AF.Exp,
                                             bias=-C_SHIFT, scale=1.0,
                                             accum_out=spart[:, k:k + 1])
                    scr10 = small_pool.tile([TC, NV // 2], BF16, tag="scr10",
                                            name="scr10")
                    lsum = small_pool.tile([TC, 1], F32, tag="lsum",
                                           name="lsum")
                    nc.scalar.activation(scr10[:], spart[:], AF.Identity,
                                         accum_out=lsum[:])
                    col = bb * NCHUNK + c
                    nc.scalar.activation(lnls_all[:, col:col + 1], lsum[:],
                                         AF.Ln)

                    # glog^T [token-row, t], then p~ = exp(glog + D) in bf16
                    gl = gl_pool.tile([NTOK, TC], F32, tag="gl", name="gl")
                    for pe in range(NPAIR):
                        nc.tensor.matmul(
                            gl[:], wx_all[:, bb, pe, :, 0:NTOK],
                            hp_t[:, pe, :, 0:TC],
                            start=(pe == 0), stop=(pe == NPAIR - 1),
                            perf_mode=DR)
                    ptc = pt_pool.tile([NTOK, TC], BF16, tag="ptc",
                                       name="ptc")
                    nc.scalar.activation(ptc[:], gl[:], AF.Exp,
                                         bias=D_SHIFT, scale=1.0)
                    nc.sync.dma_start(psweep[bb:bb + 1, :, t0:t0 + TC],
                                      ptc[:])

            # Sum_t ln lsum: one ones-matmul partition reduce for all
            # (example, chunk) columns, then a per-example strided reduce.
            lsps = lsps_pool.tile([1, BPC, NCHUNK], F32, tag="lsps",
                                  name="lsps")
            nc.tensor.matmul(lsps[:], ones125[:], lnls_all[:],
                             start=True, stop=True)
            nc.vector.tensor_reduce(llacc[:], lsps[:], axis=AX.X, op=ALU.add)

            # ================= Phase 2: F/O scan sweep =================
            masked = set(masked_j)
            qrow = psweep[:, 0, :]          # blank p~ row
            for j in range(L + 1):
                # F_j over frames [loF, hiF]; true support starts at j-1
                loF = max(0, j - 1)
                hiF = _hi(2 * j)
                if j == 0:
                    nc.vector.tensor_tensor_scan(
                        frow[:, 1:hiF + 2], qrow[:, 0:hiF + 1],
                        zrow[:, 0:hiF + 1], 1.0, ALU.mult, ALU.add)
                else:
                    oprev = orow[:, (j - 1) % 2, :]
                    if j in masked:
                        # d1 = m2*O_{j-1}(t) + (1-m2)*q(t)*O_{j-1}(t-1)
                        nc.vector.memset(vtmp[:, loF:loF + 1], 0.0)
                        nc.vector.tensor_mul(vtmp[:, loF + 1:hiF + 1],
                                             qrow[:, loF + 1:hiF + 1],
                                             oprev[:, loF:hiF])
                        nc.vector.tensor_scalar_mul(vtmp[:, loF:hiF + 1],
                                                    vtmp[:, loF:hiF + 1],
                                                    m2t[:, 1, j:j + 1])
                        nc.vector.tensor_scalar_mul(vtmp2[:, loF:hiF + 1],
                                                    oprev[:, loF:hiF + 1],
                                                    m2t[:, 0, j:j + 1])
                        nc.vector.tensor_add(vtmp[:, loF:hiF + 1],
                                             vtmp[:, loF:hiF + 1],
                                             vtmp2[:, loF:hiF + 1])
                        d1 = vtmp[:, loF:hiF + 1]
                    else:
                        d1 = oprev[:, loF:hiF + 1]
                    nc.vector.tensor_tensor_scan(
                        frow[:, loF + 1:hiF + 2], qrow[:, loF:hiF + 1],
                        d1, 0.0, ALU.mult, ALU.add)
                if j == L:
                    break
                loO = j
                hiO = _hi(2 * j + 1)
                nc.vector.tensor_tensor_scan(
                    orow[:, j % 2, loO:hiO + 1], frow[:, loO:hiO + 1],
                    psweep[:, 1 + j, loO:hiO + 1], 0.0, ALU.add, ALU.mult)

            # ================= finalize: ll = ln F_L(T-1) - llacc ==========
            lnu = ptile([BPC, 1], F32, "lnu")
            nc.scalar.activation(lnu[:], frow[:, T:T + 1], AF.Ln)
            llf = ptile([1, BPC], F32, "llf")
            nc.sync.dma_start(llf[:], lnu[:])   # [4,1] -> [1,4]
            dif = ptile([1, BPC], F32, "dif")
            nc.vector.tensor_tensor(dif[:], llf[:], llacc[:], ALU.subtract)
            tot = ptile([1, 1], F32, "tot")
            nc.vector.tensor_reduce(tot[:], dif[:], axis=AX.X, op=ALU.add)
            nc.sync.dma_start(out_d[:], tot[:])

    nc.compile()
    return nc


def prep_in_maps(hpad, W, b, ys):
    """Host-side layout prep shared by kernel() and test harnesses."""
    f8 = mybir.dt.np(FP8)
    W = np.asarray(W)
    ys = np.asarray(ys)
    # allow skip into label j (j >= 1): labels differ; j=0 has no skip source
    allow = np.ones((B, NTOK), np.float32)
    allow[:, 1:L] = (ys[:, 1:] != ys[:, :-1]).astype(np.float32)
    masked_j = tuple(sorted(
        j for j in range(1, L) if not allow[:, j].all()))
    m2 = np.stack([allow, 1.0 - allow], axis=1)       # [B, 2, NTOK]

    hpT = np.ascontiguousarray(hpad.transpose(0, 2, 1)).astype(f8)
    hpT = hpT.reshape(B, NPAIR, 2, 128, NCHUNK, TC)
    hpP = np.zeros((B, NCHUNK, 128, NPAIR, 2, TCP), dtype=f8)
    hpP[..., :TC] = hpT.transpose(0, 4, 3, 1, 2, 5)
    hpT = hpP.reshape(B, NCHUNK, 128, NPAIR * 2 * TCP)
    wtT = np.ascontiguousarray(W.T).astype(f8).reshape(NPAIR, 2, 128, V)
    # distinct tokens per example: row 0 = blank, row 1+j = label j
    toks = np.concatenate([np.zeros((B, 1), np.int64),
                           ys.astype(np.int64)], axis=1)   # [B, NTOK]
    wext = np.ascontiguousarray(
        W[toks.reshape(-1)].reshape(B, NTOK, E).transpose(0, 2, 1)
    ).astype(f8).reshape(B, NPAIR, 2, 128, NTOK)

    in_maps = []
    for c in range(NCORE):
        sl = slice(c * BPC, (c + 1) * BPC)
        in_maps.append({
            "hpt": np.ascontiguousarray(hpT[sl]),
            "wtt": wtT,
            "wxt": np.ascontiguousarray(wext[sl]),
            "m2": np.ascontiguousarray(m2[sl]),
        })
    return in_maps, masked_j


def kernel(hpad, W, b, ys):
    assert hpad.shape == (B, T, E) and W.shape == (V, E) and ys.shape == (B, L)
    assert not np.any(np.asarray(b)), "kernel assumes b == 0 (per problem spec)"

    in_maps, masked_j = prep_in_maps(hpad, W, b, ys)
    key = ("nc", masked_j)
    if key not in _cache:
        _cache[key] = _build_nc(masked_j)
    nc = _cache[key]
    _cache["nc_last"] = (nc, in_maps)

    res = run_bass_kernel_spmd(nc, in_maps, core_ids=list(range(NCORE)))
    tot = sum(float(r["out"][0, 0]) for r in res.results)
    ll_sum = tot - B * T * (C_SHIFT + D_SHIFT)
    return np.float32(-ll_sum / B)
